# revision 2
# baseline (speedup 1.0000x reference)
"""AttentionSAGEConv on 8 Trainium2 NeuronCores (Bass/Tile).

Strategy (dst-partitioned SPMD, transfer-optimized):
  - The wall-clock metric is dominated by host->device transfer over
    the axon tunnel (and NEFF load), so per-core inputs are minimized:
    each core receives only its 1/8 node slice of x (fp16, 1.6MB),
    uint16 gather indices, uint8 local-dst ids (two layouts), fp16
    edge_attr, and fp16 weights -- ~35MB total across 8 cores vs 277MB
    for the naive replicated-x layout.  The output is fp16 (cast to
    f32 on host).
  - Phase 1 (device): each core projects ONLY its local 6272 nodes to
    Q/K/V (fp16 matmuls).  K|V rows go to a local DRAM table; one
    8-core AllGather over NeuronLink replicates the full fp16 K|V
    table [8*6272, 256] (core-major rows).  Q stays SBUF-resident.
  - Phase 2 (device, per 128-node group): one indirect-DMA gather per
    128-edge block fetches K|V fp16 rows by core-major global src row
    (gathers stay on the default SWDGE queue: spreading them over
    extra named queues costs >1s of NEFF-load wall time for ~zero
    exec gain).  Q rows come from a one-hot PE expansion; the
    transposed one-hot is built directly with a partition-broadcast
    DMA of the host-transposed ldst + is_equal against a per-partition
    iota, not per-block PE transposes.  Per-edge attention edge-major
    on DVE/ACT (QK dot, device edge-bias prepass, leaky relu, exp; the
    global max subtraction cancels in the softmax and is skipped).
    Both segment-sums ride ONE one-hot matmul per block via a combined
    [V*attn | attn] rhs into f32 PSUM, then clamp+reciprocal
    normalization and the fused output
    out = relu(x @ Wm1 + agg_n @ (Wo @ Wm2) + (bo @ Wm2 + bm)).
  Measured relative error 5.8e-4 (vs 2e-2 gate).
"""

import numpy as np

N = 50000
E = 800000
IN_DIM = 128
OUT_DIM = 128
EDGE_DIM = 3
H = 4
HD = 32
SCALE = HD ** -0.5
NCORES = 8
NPC = N // NCORES          # nodes per core = 6250
G = (NPC + 127) // 128     # groups per core = 49
NPAD = G * 128             # padded nodes per core = 6272

_CACHE = {}


def _patch_tile(tile_mod, mybir, ScopedClock):
    """This walrus build allows at most ONE semaphore wait per
    instruction.  Tile's final drain aggregates many waits; replace it
    with a chain of single-wait nops, and post-split every multi-wait
    instruction the Rust scheduler produced."""
    if getattr(tile_mod.TileContext, "_ant_drain_patched", False):
        return

    def _drain_and_barrier(self, tick_clock, wait_clock):
        probe = self.nc.sync.nop(nofuse=True)
        wait_clock.add_sem_waits(probe.ins, ScopedClock({None: tick_clock.global_clock}))
        si = probe.ins.sync_info
        waits = list(si.on_wait) if si is not None and si.on_wait else []
        if len(waits) > 1:
            probe.ins.sync_info = mybir.SyncInfo(on_wait=[waits[0]], on_update=[])
            for w in waits[1:]:
                n = self.nc.sync.nop(nofuse=True)
                n.ins.sync_info = mybir.SyncInfo(on_wait=[w], on_update=[])
        self.nc.sync.drain()
        self.nc.all_engine_barrier()
        popped = self.nc._tile_sem_poison_stack.pop()
        assert popped is self._sem_poison
        self.nc.clear_and_free_semaphores(list(self.sems.allocated().values()))
        self.nc.all_engine_barrier()

    tile_mod.TileContext._drain_and_barrier = _drain_and_barrier
    tile_mod.TileContext._ant_drain_patched = True


def _split_multi_waits(nc, mybir):
    for f in nc.m.functions:
        for blk in f.blocks:
            new = []
            for inst in blk.instructions:
                si = inst.sync_info
                if si is not None and si.on_wait and len(si.on_wait) > 1:
                    waits = list(si.on_wait)
                    for k, w in enumerate(waits[:-1]):
                        new.append(mybir.InstNoOp(
                            name=f"{inst.name}-ws{k}", engine=inst.engine,
                            sync_info=mybir.SyncInfo(on_wait=[w], on_update=[]),
                            bass_nofuse=True))
                    inst.sync_info = mybir.SyncInfo(
                        on_wait=[waits[-1]], on_update=list(si.on_update or []))
                new.append(inst)
            blk.instructions = new


def _prep(edge_index, edge_attr):
    """Host-side index prep.  Returns per-core arrays with one shared
    block structure (NB blocks per group on every core/group).

    srcidx holds CORE-MAJOR global rows into the AllGathered K|V table:
    row = (src // NPC) * NPAD + (src % NPC)."""
    src = np.asarray(edge_index[0], dtype=np.int64)
    dst = np.asarray(edge_index[1], dtype=np.int64)
    src_row = ((src // NPC) * NPAD + (src % NPC)).astype(np.uint16)
    core = dst // NPC
    per_core = []
    counts_all = np.zeros((NCORES, G), dtype=np.int64)
    for c in range(NCORES):
        sel = np.nonzero(core == c)[0]
        d_loc = dst[sel] - c * NPC
        order = np.argsort(d_loc, kind="stable")
        sel = sel[order]
        d_loc = d_loc[order]
        counts = np.bincount(d_loc // 128, minlength=G)
        counts_all[c] = counts
        per_core.append((sel, d_loc, counts))

    # per-group block count = max over cores (SPMD needs per-g uniformity)
    nbs = ((counts_all.max(axis=0) + 127) // 128).astype(int)
    nbs = np.maximum(nbs, 1)
    b0s = np.concatenate([[0], np.cumsum(nbs)]).astype(int)
    B = int(b0s[-1])
    ins = []
    for c in range(NCORES):
        sel, d_loc, counts = per_core[c]
        srcidx = np.zeros((128, B), dtype=np.uint16)
        ldst = np.full((128, B), 255, dtype=np.uint8)
        ea = np.zeros((128, B, 3), dtype=np.float16)
        starts = np.concatenate([[0], np.cumsum(counts)])
        for g in range(G):
            e0, e1 = starts[g], starts[g + 1]
            idxs = sel[e0:e1]
            k = e1 - e0
            slot = np.arange(k)
            b = b0s[g] + slot // 128
            p = slot % 128
            srcidx[p, b] = src_row[idxs]
            ldst[p, b] = (d_loc[e0:e1] - g * 128).astype(np.uint8)
            ea[p, b, :] = edge_attr[idxs]
        ldstT = np.ascontiguousarray(ldst.T).reshape(1, B * 128)
        ins.append(dict(srcidx=srcidx, ldst=ldst, ldstT=ldstT, eat=ea))
    return ins, nbs, b0s, B


def _build(nbs, b0s, B, bufs2=3, bufsps=2):
    import concourse.bass as bass
    import concourse.mybir as mybir
    import concourse.tile as tile
    from concourse.vector_clock import ScopedClock
    from concourse.masks import make_identity

    _patch_tile(tile, mybir, ScopedClock)
    f32 = mybir.dt.float32
    f16 = mybir.dt.float16
    AL = mybir.AluOpType

    nc = bass.Bass(target_bir_lowering=False, num_swdge_queues=4)
    # ---- inputs (per core) ----
    xtl = nc.dram_tensor("xtl", [128, NPAD], f16, kind="ExternalInput")
    Wqkv = nc.dram_tensor("Wqkv", [128, 384], f16, kind="ExternalInput")
    Wm1 = nc.dram_tensor("Wm1", [128, 128], f16, kind="ExternalInput")
    Wm2 = nc.dram_tensor("Wm2", [128, 128], f16, kind="ExternalInput")
    WoT = nc.dram_tensor("WoT", [128, 128], f16, kind="ExternalInput")
    boc = nc.dram_tensor("boc", [128, 1], f16, kind="ExternalInput")
    bmr = nc.dram_tensor("bmr", [1, 128], f32, kind="ExternalInput")
    srcidx = nc.dram_tensor("srcidx", [128, B], mybir.dt.uint16, kind="ExternalInput")
    ldst = nc.dram_tensor("ldst", [128, B], mybir.dt.uint8, kind="ExternalInput")
    ldstT = nc.dram_tensor("ldstT", [1, B * 128], mybir.dt.uint8, kind="ExternalInput")
    eat = nc.dram_tensor("eat", [128, B, 3], f16, kind="ExternalInput")
    Wef = nc.dram_tensor("Wef", [1, 12], f32, kind="ExternalInput")
    out = nc.dram_tensor("out", [NPC, 128], f16, kind="ExternalOutput")
    kvl = nc.dram_tensor("kvl", [NPAD, 256], f16)            # local K|V
    kvt = nc.dram_tensor("kvt", [NCORES * NPAD, 256], f16,
                         addr_space="Shared")                # gathered K|V

    with tile.TileContext(nc) as tc:
        with tc.tile_pool(name="const", bufs=1) as cpool, \
             tc.tile_pool(name="sb", bufs=3) as sb, \
             tc.tile_pool(name="sb2", bufs=bufs2) as sb2, \
             tc.tile_pool(name="ps", bufs=bufsps, space="PSUM") as ps, \
             tc.tile_pool(name="psb", bufs=1, space="PSUM") as psb, \
             tc.tile_pool(name="ps1", bufs=2, space="PSUM") as ps1:

            # ---------- constants / setup ----------
            idt = cpool.tile([128, 128], f32)
            make_identity(nc, idt[:])
            idt16 = cpool.tile([128, 128], f16)
            make_identity(nc, idt16[:])
            iota16 = cpool.tile([128, 128], f16)
            nc.gpsimd.iota(iota16[:], pattern=[[1, 128]], base=0,
                           channel_multiplier=0,
                           allow_small_or_imprecise_dtypes=True)
            iotac = cpool.tile([128, 1], f16)
            nc.gpsimd.iota(iotac[:], pattern=[[1, 1]], base=0,
                           channel_multiplier=1,
                           allow_small_or_imprecise_dtypes=True)
            xtl_sb = cpool.tile([128, NPAD], f16)
            nc.sync.dma_start(out=xtl_sb[:], in_=xtl[:])
            wqkv_sb = cpool.tile([128, 384], f16)
            nc.sync.dma_start(out=wqkv_sb[:], in_=Wqkv[:])
            wm1_sb = cpool.tile([128, 128], f16)
            nc.sync.dma_start(out=wm1_sb[:], in_=Wm1[:])
            wm2_sb = cpool.tile([128, 128], f16)
            nc.sync.dma_start(out=wm2_sb[:], in_=Wm2[:])
            woT_sb = cpool.tile([128, 128], f16)
            nc.sync.dma_start(out=woT_sb[:], in_=WoT[:])
            bo_sb = cpool.tile([128, 1], f16)
            nc.sync.dma_start(out=bo_sb[:], in_=boc[:])
            bm_sb = cpool.tile([1, 128], f32)
            nc.sync.dma_start(out=bm_sb[:], in_=bmr[:])
            ones1 = cpool.tile([1, 128], f16)
            nc.gpsimd.memset(ones1[:], 1.0)
            ones1f = cpool.tile([1, 128], f32)
            nc.gpsimd.memset(ones1f[:], 1.0)
            wef_sb = cpool.tile([1, 12], f32)
            nc.sync.dma_start(out=wef_sb[:], in_=Wef[:])

            # integer/bias inputs -> working dtypes
            srci_u16 = cpool.tile([128, B], mybir.dt.uint16)
            nc.sync.dma_start(out=srci_u16[:], in_=srcidx[:])
            srcidx_sb = cpool.tile([128, B], mybir.dt.int32)
            nc.vector.tensor_copy(out=srcidx_sb[:], in_=srci_u16[:])
            ldst_u8 = cpool.tile([128, B], mybir.dt.uint8)
            nc.sync.dma_start(out=ldst_u8[:], in_=ldst[:])
            ldst_sb = cpool.tile([128, B], f16)
            nc.vector.tensor_copy(out=ldst_sb[:], in_=ldst_u8[:])
            ea16 = cpool.tile([128, B, 3], f16)
            nc.sync.dma_start(out=ea16[:], in_=eat[:])
            eaf = cpool.tile([128, B, 3], f32)
            nc.scalar.copy(out=eaf[:], in_=ea16[:])

            # W2 = Wo @ Wm2  (WoT supplied pre-transposed from host)
            pw2 = ps1.tile([128, 384], f32, tag="p1p")
            nc.tensor.matmul(out=pw2[:, :128], lhsT=woT_sb[:], rhs=wm2_sb[:], start=True, stop=True)
            w2_sb = cpool.tile([128, 128], f16)
            nc.scalar.copy(out=w2_sb[:], in_=pw2[:, :128])

            # b2 = bo @ Wm2 + bm   [1, 128] fp16
            pb2 = ps1.tile([128, 384], f32, tag="p1p")
            nc.tensor.matmul(out=pb2[:1, :128], lhsT=bo_sb[:], rhs=wm2_sb[:], start=True, stop=True)
            b2_sb = cpool.tile([1, 128], f16)
            nc.vector.tensor_tensor(out=b2_sb[:], in0=pb2[:1, :128], in1=bm_sb[:], op=AL.add)

            # We replicated to all partitions: [128, 12]
            pwe = ps1.tile([128, 384], f32, tag="p1p")
            nc.tensor.matmul(out=pwe[:, :12], lhsT=ones1f[:], rhs=wef_sb[:], start=True, stop=True)
            we_rep = cpool.tile([128, 12], f32)
            nc.scalar.copy(out=we_rep[:], in_=pwe[:, :12])

            # edge bias prepass: bias_all [128, B, 4]
            bias_all = cpool.tile([128, B, 4], f32)
            for h in range(H):
                nc.vector.tensor_scalar(
                    out=bias_all[:, :, h], in0=eaf[:, :, 0],
                    scalar1=we_rep[:, 0 * 4 + h:0 * 4 + h + 1], scalar2=None,
                    op0=AL.mult)
                for j in (1, 2):
                    nc.vector.scalar_tensor_tensor(
                        out=bias_all[:, :, h], in0=eaf[:, :, j],
                        scalar=we_rep[:, j * 4 + h:j * 4 + h + 1],
                        in1=bias_all[:, :, h], op0=AL.mult, op1=AL.add)

            # ---------- phase 1: local Q/K/V; K|V -> DRAM, Q stays SBUF ----------
            qtab = cpool.tile([128, G, 128], f16)
            for g in range(G):
                pq = ps1.tile([128, 384], f32, tag="p1p")
                nc.tensor.matmul(out=pq[:],
                                 lhsT=xtl_sb[:, g * 128:(g + 1) * 128],
                                 rhs=wqkv_sb[:], start=True, stop=True)
                kv16 = sb.tile([128, 256], f16, tag="p1o")
                if g % 2 == 0:
                    nc.vector.tensor_copy(out=qtab[:, g, :], in_=pq[:, 0:128])
                    nc.scalar.copy(out=kv16[:], in_=pq[:, 128:384])
                else:
                    nc.scalar.copy(out=qtab[:, g, :], in_=pq[:, 0:128])
                    nc.vector.tensor_copy(out=kv16[:], in_=pq[:, 128:384])
                nc.sync.dma_start(out=kvl[g * 128:(g + 1) * 128, :], in_=kv16[:])

            # ---------- AllGather K|V across the 8 cores ----------
            nc.gpsimd.collective_compute(
                "AllGather", mybir.AluOpType.bypass,
                replica_groups=[list(range(NCORES))],
                ins=[kvl.ap().opt()], outs=[kvt.ap().opt()])

            # ---------- phase 2 ----------
            NBMAX = int(max(nbs))
            for g in range(G):
                NB = int(nbs[g])
                b0 = int(b0s[g])
                rows = min(128, NPC - g * 128)

                # gathered per-edge K|V for the whole group
                kvg = sb2.tile([128, NBMAX, 256], f16, tag="kvg")
                for b in range(NB):
                    nc.gpsimd.indirect_dma_start(
                        out=kvg[:, b, :], out_offset=None, in_=kvt[:],
                        in_offset=bass.IndirectOffsetOnAxis(
                            ap=srcidx_sb[:, b0 + b:b0 + b + 1], axis=0))

                # one-hot [128e, NB, 128n] for scatter; transposed one-hot
                # [128n, NB, 128e] for Q expansion (built from a broadcast
                # DMA of the host-transposed ldst, not per-block PE
                # transposes)
                oh = sb2.tile([128, NBMAX, 128], f16, tag="oh")
                nc.vector.tensor_tensor(
                    out=oh[:, :NB, :],
                    in0=ldst_sb[:, b0:b0 + NB, None].to_broadcast([128, NB, 128]),
                    in1=iota16[:, None, :].to_broadcast([128, NB, 128]),
                    op=AL.is_equal)
                repu = sb.tile([128, NBMAX * 128], mybir.dt.uint8, tag="repu")
                nc.sync.dma_start(
                    out=repu[:, :NB * 128],
                    in_=ldstT[:, b0 * 128:(b0 + NB) * 128].to_broadcast(
                        [128, NB * 128]))
                rep16 = sb.tile([128, NBMAX * 128], f16, tag="rep16")
                nc.vector.tensor_copy(out=rep16[:, :NB * 128],
                                      in_=repu[:, :NB * 128])
                ohT = sb2.tile([128, NBMAX, 128], f16, tag="ohT")
                nc.vector.tensor_tensor(
                    out=ohT[:, :NB, :],
                    in0=rep16[:, :NB * 128].rearrange("p (b e) -> p b e", e=128),
                    in1=iotac[:, :, None].to_broadcast([128, NB, 128]),
                    op=AL.is_equal)
                pk = sb2.tile([128, NBMAX, 128], f32, tag="pk")
                for b4 in range(0, NB, 4):
                    nb4 = min(4, NB - b4)
                    pqe = psb.tile([128, 4, 128], f32, tag="pqe")
                    for j in range(nb4):
                        b = b4 + j
                        nc.tensor.matmul(out=pqe[:, j, :], lhsT=ohT[:, b, :],
                                         rhs=qtab[:, g, :], start=True, stop=True)
                    nc.vector.tensor_tensor(out=pk[:, b4:b4 + nb4, :],
                                            in0=pqe[:, :nb4, :],
                                            in1=kvg[:, b4:b4 + nb4, 0:128], op=AL.mult)
                attnf = sb2.tile([128, NBMAX, 4], f32, tag="attnf")
                nc.vector.tensor_reduce(
                    out=attnf[:, :NB, :],
                    in_=pk[:, :NB, :].rearrange("p b (h d) -> p (b h) d", d=32),
                    axis=mybir.AxisListType.X, op=AL.add)
                # z = attn*SCALE + bias ; leaky = max(z, 0.2 z) ; exp
                nc.vector.scalar_tensor_tensor(
                    out=attnf[:, :NB, :], in0=attnf[:, :NB, :], scalar=SCALE,
                    in1=bias_all[:, b0:b0 + NB, :], op0=AL.mult, op1=AL.add)
                nc.vector.scalar_tensor_tensor(
                    out=attnf[:, :NB, :], in0=attnf[:, :NB, :], scalar=0.2,
                    in1=attnf[:, :NB, :], op0=AL.mult, op1=AL.max)
                # wva = [V * attn | attn]  (one combined rhs so the two
                # segment-sums collapse into one matmul per block)
                wva = sb2.tile([128, NBMAX, 132], f16, tag="wva")
                nc.scalar.activation(out=wva[:, :NB, 128:132],
                                     in_=attnf[:, :NB, :],
                                     func=mybir.ActivationFunctionType.Exp)
                nc.vector.tensor_tensor(
                    out=wva[:, :NB, 0:128].rearrange("p b (h d) -> p b h d", d=32),
                    in0=kvg[:, :NB, 128:256].rearrange("p b (h d) -> p b h d", d=32),
                    in1=wva[:, :NB, 128:132, None].to_broadcast([128, NB, 4, 32]),
                    op=AL.mult)

                # scatter to nodes: pagg2 = [agg | attn_sum]
                pagg2 = ps.tile([128, 132], f32, tag="pagg")
                for b in range(NB):
                    nc.tensor.matmul(out=pagg2[:], lhsT=oh[:, b, :], rhs=wva[:, b, :],
                                     start=(b == 0), stop=(b == NB - 1))

                # normalize
                sums = sb.tile([128, 4], f32, tag="sums")
                nc.vector.tensor_scalar(out=sums[:], in0=pagg2[:, 128:132],
                                        scalar1=1e-12,
                                        scalar2=None, op0=AL.max)
                rec = sb.tile([128, 4], f32, tag="rec")
                nc.vector.reciprocal(out=rec[:], in_=sums[:])
                aggn = sb.tile([128, 128], f32, tag="aggn")
                nc.vector.tensor_tensor(
                    out=aggn[:].rearrange("p (h d) -> p h d", d=32),
                    in0=pagg2[:, 0:128].rearrange("p (h d) -> p h d", d=32),
                    in1=rec[:, :, None].to_broadcast([128, 4, 32]), op=AL.mult)
                ptr = psb.tile([128, 128], f32, tag="ptrpo")
                nc.tensor.transpose(out=ptr[:], in_=aggn[:], identity=idt[:])
                aggnT = sb.tile([128, 128], f16, tag="aggnT")
                nc.scalar.copy(out=aggnT[:], in_=ptr[:])

                # out = relu(x@Wm1 + aggn@W2 + b2)
                po = psb.tile([128, 128], f32, tag="ptrpo")
                nc.tensor.matmul(out=po[:], lhsT=xtl_sb[:, g * 128:(g + 1) * 128],
                                 rhs=wm1_sb[:], start=True, stop=False)
                nc.tensor.matmul(out=po[:], lhsT=aggnT[:], rhs=w2_sb[:],
                                 start=False, stop=False)
                nc.tensor.matmul(out=po[:], lhsT=ones1[:], rhs=b2_sb[:],
                                 start=False, stop=True)
                osb = sb.tile([128, 128], f16, tag="osb")
                nc.scalar.activation(out=osb[:], in_=po[:],
                                     func=mybir.ActivationFunctionType.Relu)
                nc.sync.dma_start(out=out[g * 128:g * 128 + rows, :],
                                  in_=osb[:rows, :])

    _split_multi_waits(nc, mybir)
    return nc


def kernel(x, edge_index, edge_attr, Wq, Wk, Wv, We, Wo, bo, Wm, bm):
    from concourse.bass_utils import run_bass_kernel_spmd

    x = np.asarray(x, dtype=np.float32)
    edge_attr = np.asarray(edge_attr, dtype=np.float16)
    per_core, nbs, b0s, B = _prep(np.asarray(edge_index), edge_attr)

    key = (tuple(nbs.tolist()), B)
    if key not in _CACHE:
        _CACHE[key] = _build(nbs, b0s, B)
    nc = _CACHE[key]

    xT = x.T  # [128, N]
    Wqkv = np.concatenate(
        [np.asarray(Wq, np.float32), np.asarray(Wk, np.float32),
         np.asarray(Wv, np.float32)], axis=1).astype(np.float16)
    Wm = np.asarray(Wm, np.float32)
    common = dict(
        Wqkv=np.ascontiguousarray(Wqkv),
        Wm1=np.ascontiguousarray(Wm[:128]).astype(np.float16),
        Wm2=np.ascontiguousarray(Wm[128:]).astype(np.float16),
        WoT=np.ascontiguousarray(np.asarray(Wo, np.float32).T).astype(np.float16),
        boc=np.asarray(bo, np.float32).reshape(128, 1).astype(np.float16),
        bmr=np.asarray(bm, np.float32).reshape(1, 128),
        Wef=np.asarray(We, np.float32).reshape(1, 12),
    )
    in_maps = []
    for c in range(NCORES):
        m = dict(common)
        cols = np.zeros((128, NPAD), dtype=np.float16)
        cols[:, :NPC] = xT[:, c * NPC:(c + 1) * NPC]
        m["xtl"] = cols
        m.update(per_core[c])
        in_maps.append(m)

    import time as _time
    _t0 = _time.perf_counter()
    res = run_bass_kernel_spmd(nc, in_maps, core_ids=list(range(NCORES)))
    global _LAST_RESULTS, _LAST_RUN_NS
    _LAST_RUN_NS = int((_time.perf_counter() - _t0) * 1e9)
    _LAST_RESULTS = res
    outs = [res.results[c]["out"] for c in range(NCORES)]
    return np.concatenate(outs, axis=0).astype(np.float32)


_LAST_RESULTS = None
_LAST_RUN_NS = None


# revision 3
# speedup vs baseline: 1.1572x; 1.1572x over previous
"""AttentionSAGEConv on 8 Trainium2 NeuronCores (Bass/Tile).

Strategy (dst-partitioned SPMD, transfer-optimized):
  - The wall-clock metric is dominated by host->device transfer over
    the axon tunnel (and NEFF load), so per-core inputs are minimized:
    each core receives only its 1/8 node slice of x (fp16, 1.6MB),
    uint16 gather indices, uint8 local-dst ids (two layouts), fp16
    edge_attr, and fp16 weights -- ~35MB total across 8 cores vs 277MB
    for the naive replicated-x layout.  The output is fp16 (cast to
    f32 on host).
  - Phase 1 (device): each core projects ONLY its local 6272 nodes to
    Q/K/V (fp16 matmuls).  K|V rows go to a local DRAM table; one
    8-core AllGather over NeuronLink replicates the full fp16 K|V
    table [8*6272, 256] (core-major rows).  Q stays SBUF-resident.
  - Phase 2 (device, per 128-node group): one indirect-DMA gather per
    128-edge block fetches K|V fp16 rows by core-major global src row
    (gathers stay on the default SWDGE queue: spreading them over
    extra named queues costs >1s of NEFF-load wall time for ~zero
    exec gain).  Q rows come from a one-hot PE expansion; the
    transposed one-hot is built directly with a partition-broadcast
    DMA of the host-transposed ldst + is_equal against a per-partition
    iota, not per-block PE transposes.  Per-edge attention edge-major
    on DVE/ACT (QK dot, device edge-bias prepass, leaky relu, exp; the
    global max subtraction cancels in the softmax and is skipped).
    Both segment-sums ride ONE one-hot matmul per block via a combined
    [V*attn | attn] rhs into f32 PSUM, then clamp+reciprocal
    normalization and the fused output
    out = relu(x @ Wm1 + agg_n @ (Wo @ Wm2) + (bo @ Wm2 + bm)).
  Measured relative error 5.8e-4 (vs 2e-2 gate).
"""

import numpy as np

N = 50000
E = 800000
IN_DIM = 128
OUT_DIM = 128
EDGE_DIM = 3
H = 4
HD = 32
SCALE = HD ** -0.5
NCORES = 8
NPC = N // NCORES          # nodes per core = 6250
G = (NPC + 127) // 128     # groups per core = 49
NPAD = G * 128             # padded nodes per core = 6272

_CACHE = {}


def _patch_tile(tile_mod, mybir, ScopedClock):
    """This walrus build allows at most ONE semaphore wait per
    instruction.  Tile's final drain aggregates many waits; replace it
    with a chain of single-wait nops, and post-split every multi-wait
    instruction the Rust scheduler produced."""
    if getattr(tile_mod.TileContext, "_ant_drain_patched", False):
        return

    def _drain_and_barrier(self, tick_clock, wait_clock):
        probe = self.nc.sync.nop(nofuse=True)
        wait_clock.add_sem_waits(probe.ins, ScopedClock({None: tick_clock.global_clock}))
        si = probe.ins.sync_info
        waits = list(si.on_wait) if si is not None and si.on_wait else []
        if len(waits) > 1:
            probe.ins.sync_info = mybir.SyncInfo(on_wait=[waits[0]], on_update=[])
            for w in waits[1:]:
                n = self.nc.sync.nop(nofuse=True)
                n.ins.sync_info = mybir.SyncInfo(on_wait=[w], on_update=[])
        self.nc.sync.drain()
        self.nc.all_engine_barrier()
        popped = self.nc._tile_sem_poison_stack.pop()
        assert popped is self._sem_poison
        self.nc.clear_and_free_semaphores(list(self.sems.allocated().values()))
        self.nc.all_engine_barrier()

    tile_mod.TileContext._drain_and_barrier = _drain_and_barrier
    tile_mod.TileContext._ant_drain_patched = True


def _split_multi_waits(nc, mybir):
    for f in nc.m.functions:
        for blk in f.blocks:
            new = []
            for inst in blk.instructions:
                si = inst.sync_info
                if si is not None and si.on_wait and len(si.on_wait) > 1:
                    waits = list(si.on_wait)
                    for k, w in enumerate(waits[:-1]):
                        new.append(mybir.InstNoOp(
                            name=f"{inst.name}-ws{k}", engine=inst.engine,
                            sync_info=mybir.SyncInfo(on_wait=[w], on_update=[]),
                            bass_nofuse=True))
                    inst.sync_info = mybir.SyncInfo(
                        on_wait=[waits[-1]], on_update=list(si.on_update or []))
                new.append(inst)
            blk.instructions = new


def _prep(edge_index, edge_attr):
    """Host-side index prep.  Returns per-core arrays with one shared
    block structure (NB blocks per group on every core/group).

    srcidx holds CORE-MAJOR global rows into the AllGathered K|V table:
    row = (src // NPC) * NPAD + (src % NPC)."""
    src = np.asarray(edge_index[0], dtype=np.int64)
    dst = np.asarray(edge_index[1], dtype=np.int64)
    src_row = ((src // NPC) * NPAD + (src % NPC)).astype(np.uint16)
    core = dst // NPC
    per_core = []
    counts_all = np.zeros((NCORES, G), dtype=np.int64)
    for c in range(NCORES):
        sel = np.nonzero(core == c)[0]
        d_loc = dst[sel] - c * NPC
        order = np.argsort(d_loc, kind="stable")
        sel = sel[order]
        d_loc = d_loc[order]
        counts = np.bincount(d_loc // 128, minlength=G)
        counts_all[c] = counts
        per_core.append((sel, d_loc, counts))

    # per-group block count = max over cores (SPMD needs per-g uniformity)
    nbs = ((counts_all.max(axis=0) + 127) // 128).astype(int)
    nbs = np.maximum(nbs, 1)
    b0s = np.concatenate([[0], np.cumsum(nbs)]).astype(int)
    B = int(b0s[-1])
    ins = []
    for c in range(NCORES):
        sel, d_loc, counts = per_core[c]
        srcidx = np.zeros((128, B), dtype=np.uint16)
        ldst = np.full((128, B), 255, dtype=np.uint8)
        ea = np.zeros((128, B, 3), dtype=np.float16)
        starts = np.concatenate([[0], np.cumsum(counts)])
        for g in range(G):
            e0, e1 = starts[g], starts[g + 1]
            idxs = sel[e0:e1]
            k = e1 - e0
            slot = np.arange(k)
            b = b0s[g] + slot // 128
            p = slot % 128
            srcidx[p, b] = src_row[idxs]
            ldst[p, b] = (d_loc[e0:e1] - g * 128).astype(np.uint8)
            ea[p, b, :] = edge_attr[idxs]
        ldstT = np.ascontiguousarray(ldst.T).reshape(1, B * 128)
        ins.append(dict(srcidx=srcidx, ldst=ldst, ldstT=ldstT, eat=ea))
    return ins, nbs, b0s, B


def _build(nbs, b0s, B, bufs2=3, bufsps=2):
    import concourse.bass as bass
    import concourse.mybir as mybir
    import concourse.tile as tile
    from concourse.vector_clock import ScopedClock
    from concourse.masks import make_identity

    _patch_tile(tile, mybir, ScopedClock)
    f32 = mybir.dt.float32
    f16 = mybir.dt.float16
    AL = mybir.AluOpType

    nc = bass.Bass(target_bir_lowering=False, num_swdge_queues=4)
    # ---- inputs (per core) ----
    xtl = nc.dram_tensor("xtl", [128, NPAD], f16, kind="ExternalInput")
    Wqkv = nc.dram_tensor("Wqkv", [128, 384], f16, kind="ExternalInput")
    Wm1 = nc.dram_tensor("Wm1", [128, 128], f16, kind="ExternalInput")
    Wm2 = nc.dram_tensor("Wm2", [128, 128], f16, kind="ExternalInput")
    WoT = nc.dram_tensor("WoT", [128, 128], f16, kind="ExternalInput")
    boc = nc.dram_tensor("boc", [128, 1], f16, kind="ExternalInput")
    bmr = nc.dram_tensor("bmr", [1, 128], f32, kind="ExternalInput")
    srcidx = nc.dram_tensor("srcidx", [128, B], mybir.dt.uint16, kind="ExternalInput")
    ldst = nc.dram_tensor("ldst", [128, B], mybir.dt.uint8, kind="ExternalInput")
    ldstT = nc.dram_tensor("ldstT", [1, B * 128], mybir.dt.uint8, kind="ExternalInput")
    eat = nc.dram_tensor("eat", [128, B, 3], f16, kind="ExternalInput")
    Wef = nc.dram_tensor("Wef", [1, 12], f32, kind="ExternalInput")
    out = nc.dram_tensor("out", [NPC, 128], f16, kind="ExternalOutput")
    kvl = nc.dram_tensor("kvl", [NPAD, 256], f16)            # local K|V
    kvt = nc.dram_tensor("kvt", [NCORES * NPAD, 256], f16,
                         addr_space="Shared")                # gathered K|V

    with tile.TileContext(nc) as tc:
        with tc.tile_pool(name="const", bufs=1) as cpool, \
             tc.tile_pool(name="sb", bufs=3) as sb, \
             tc.tile_pool(name="sb2", bufs=bufs2) as sb2, \
             tc.tile_pool(name="ps", bufs=bufsps, space="PSUM") as ps, \
             tc.tile_pool(name="psb", bufs=1, space="PSUM") as psb, \
             tc.tile_pool(name="ps1", bufs=2, space="PSUM") as ps1:

            # ---------- constants / setup ----------
            idt = cpool.tile([128, 128], f32)
            make_identity(nc, idt[:])
            iota16 = cpool.tile([128, 128], f16)
            nc.gpsimd.iota(iota16[:], pattern=[[1, 128]], base=0,
                           channel_multiplier=0,
                           allow_small_or_imprecise_dtypes=True)
            iotac = cpool.tile([128, 1], f16)
            nc.gpsimd.iota(iotac[:], pattern=[[1, 1]], base=0,
                           channel_multiplier=1,
                           allow_small_or_imprecise_dtypes=True)
            xtl_sb = cpool.tile([128, NPAD], f16)
            nc.sync.dma_start(out=xtl_sb[:], in_=xtl[:])
            wqkv_sb = cpool.tile([128, 384], f16)
            nc.sync.dma_start(out=wqkv_sb[:], in_=Wqkv[:])
            wm1_sb = cpool.tile([128, 128], f16)
            nc.sync.dma_start(out=wm1_sb[:], in_=Wm1[:])
            wm2_sb = cpool.tile([128, 128], f16)
            nc.sync.dma_start(out=wm2_sb[:], in_=Wm2[:])
            woT_sb = cpool.tile([128, 128], f16)
            nc.sync.dma_start(out=woT_sb[:], in_=WoT[:])
            bo_sb = cpool.tile([128, 1], f16)
            nc.sync.dma_start(out=bo_sb[:], in_=boc[:])
            bm_sb = cpool.tile([1, 128], f32)
            nc.sync.dma_start(out=bm_sb[:], in_=bmr[:])
            ones1 = cpool.tile([1, 128], f16)
            nc.gpsimd.memset(ones1[:], 1.0)
            ones1f = cpool.tile([1, 128], f32)
            nc.gpsimd.memset(ones1f[:], 1.0)
            wef_sb = cpool.tile([1, 12], f32)
            nc.sync.dma_start(out=wef_sb[:], in_=Wef[:])

            # integer/bias inputs -> working dtypes
            srci_u16 = cpool.tile([128, B], mybir.dt.uint16)
            nc.sync.dma_start(out=srci_u16[:], in_=srcidx[:])
            srcidx_sb = cpool.tile([128, B], mybir.dt.int32)
            nc.vector.tensor_copy(out=srcidx_sb[:], in_=srci_u16[:])
            ldst_u8 = cpool.tile([128, B], mybir.dt.uint8)
            nc.sync.dma_start(out=ldst_u8[:], in_=ldst[:])
            ldst_sb = cpool.tile([128, B], f16)
            nc.vector.tensor_copy(out=ldst_sb[:], in_=ldst_u8[:])
            ea16 = cpool.tile([128, B, 3], f16)
            nc.sync.dma_start(out=ea16[:], in_=eat[:])
            eaf = cpool.tile([128, B, 3], f32)
            nc.scalar.copy(out=eaf[:], in_=ea16[:])

            # W2 = Wo @ Wm2  (WoT supplied pre-transposed from host)
            pw2 = ps1.tile([128, 384], f32, tag="p1p")
            nc.tensor.matmul(out=pw2[:, :128], lhsT=woT_sb[:], rhs=wm2_sb[:], start=True, stop=True)
            w2_sb = cpool.tile([128, 128], f16)
            nc.scalar.copy(out=w2_sb[:], in_=pw2[:, :128])

            # b2 = bo @ Wm2 + bm   [1, 128] fp16
            pb2 = ps1.tile([128, 384], f32, tag="p1p")
            nc.tensor.matmul(out=pb2[:1, :128], lhsT=bo_sb[:], rhs=wm2_sb[:], start=True, stop=True)
            b2_sb = cpool.tile([1, 128], f16)
            nc.vector.tensor_tensor(out=b2_sb[:], in0=pb2[:1, :128], in1=bm_sb[:], op=AL.add)

            # We replicated to all partitions: [128, 12]
            pwe = ps1.tile([128, 384], f32, tag="p1p")
            nc.tensor.matmul(out=pwe[:, :12], lhsT=ones1f[:], rhs=wef_sb[:], start=True, stop=True)
            we_rep = cpool.tile([128, 12], f32)
            nc.scalar.copy(out=we_rep[:], in_=pwe[:, :12])

            # edge bias prepass: bias_all [128, B, 4]
            bias_all = cpool.tile([128, B, 4], f32)
            for h in range(H):
                nc.vector.tensor_scalar(
                    out=bias_all[:, :, h], in0=eaf[:, :, 0],
                    scalar1=we_rep[:, 0 * 4 + h:0 * 4 + h + 1], scalar2=None,
                    op0=AL.mult)
                for j in (1, 2):
                    nc.vector.scalar_tensor_tensor(
                        out=bias_all[:, :, h], in0=eaf[:, :, j],
                        scalar=we_rep[:, j * 4 + h:j * 4 + h + 1],
                        in1=bias_all[:, :, h], op0=AL.mult, op1=AL.add)

            # ---------- phase 1: local Q/K/V; K|V -> DRAM, Q stays SBUF ----------
            qtab = cpool.tile([128, G, 128], f16)
            for g in range(G):
                pq = ps1.tile([128, 384], f32, tag="p1p")
                nc.tensor.matmul(out=pq[:],
                                 lhsT=xtl_sb[:, g * 128:(g + 1) * 128],
                                 rhs=wqkv_sb[:], start=True, stop=True)
                kv16 = sb.tile([128, 256], f16, tag="p1o")
                if g % 2 == 0:
                    nc.vector.tensor_copy(out=qtab[:, g, :], in_=pq[:, 0:128])
                    nc.scalar.copy(out=kv16[:], in_=pq[:, 128:384])
                else:
                    nc.scalar.copy(out=qtab[:, g, :], in_=pq[:, 0:128])
                    nc.vector.tensor_copy(out=kv16[:], in_=pq[:, 128:384])
                nc.sync.dma_start(out=kvl[g * 128:(g + 1) * 128, :], in_=kv16[:])

            # ---------- AllGather K|V across the 8 cores ----------
            nc.gpsimd.collective_compute(
                "AllGather", mybir.AluOpType.bypass,
                replica_groups=[list(range(NCORES))],
                ins=[kvl.ap().opt()], outs=[kvt.ap().opt()])

            # ---------- phase 2 ----------
            NBMAX = int(max(nbs))
            for g in range(G):
                NB = int(nbs[g])
                b0 = int(b0s[g])
                rows = min(128, NPC - g * 128)

                # gathered per-edge K|V for the whole group
                kvg = sb2.tile([128, NBMAX, 256], f16, tag="kvg")
                for b in range(NB):
                    nc.gpsimd.indirect_dma_start(
                        out=kvg[:, b, :], out_offset=None, in_=kvt[:],
                        in_offset=bass.IndirectOffsetOnAxis(
                            ap=srcidx_sb[:, b0 + b:b0 + b + 1], axis=0))

                # one-hot [128e, NB, 128n] for scatter; transposed one-hot
                # [128n, NB, 128e] for Q expansion (built from a broadcast
                # DMA of the host-transposed ldst, not per-block PE
                # transposes)
                oh = sb2.tile([128, NBMAX, 128], f16, tag="oh")
                nc.vector.tensor_tensor(
                    out=oh[:, :NB, :],
                    in0=ldst_sb[:, b0:b0 + NB, None].to_broadcast([128, NB, 128]),
                    in1=iota16[:, None, :].to_broadcast([128, NB, 128]),
                    op=AL.is_equal)
                repu = sb.tile([128, NBMAX * 128], mybir.dt.uint8, tag="repu")
                nc.sync.dma_start(
                    out=repu[:, :NB * 128],
                    in_=ldstT[:, b0 * 128:(b0 + NB) * 128].to_broadcast(
                        [128, NB * 128]))
                rep16 = sb.tile([128, NBMAX * 128], f16, tag="rep16")
                nc.vector.tensor_copy(out=rep16[:, :NB * 128],
                                      in_=repu[:, :NB * 128])
                ohT = sb2.tile([128, NBMAX, 128], f16, tag="ohT")
                nc.vector.tensor_tensor(
                    out=ohT[:, :NB, :],
                    in0=rep16[:, :NB * 128].rearrange("p (b e) -> p b e", e=128),
                    in1=iotac[:, :, None].to_broadcast([128, NB, 128]),
                    op=AL.is_equal)
                pk = sb2.tile([128, NBMAX, 128], f32, tag="pk")
                for b4 in range(0, NB, 4):
                    nb4 = min(4, NB - b4)
                    pqe = psb.tile([128, 4, 128], f32, tag="pqe")
                    for j in range(nb4):
                        b = b4 + j
                        nc.tensor.matmul(out=pqe[:, j, :], lhsT=ohT[:, b, :],
                                         rhs=qtab[:, g, :], start=True, stop=True)
                    nc.vector.tensor_tensor(out=pk[:, b4:b4 + nb4, :],
                                            in0=pqe[:, :nb4, :],
                                            in1=kvg[:, b4:b4 + nb4, 0:128], op=AL.mult)
                attnf = sb2.tile([128, NBMAX, 4], f32, tag="attnf")
                nc.vector.tensor_reduce(
                    out=attnf[:, :NB, :],
                    in_=pk[:, :NB, :].rearrange("p b (h d) -> p (b h) d", d=32),
                    axis=mybir.AxisListType.X, op=AL.add)
                # z = attn*SCALE + bias ; leaky = max(z, 0.2 z) ; exp
                nc.vector.scalar_tensor_tensor(
                    out=attnf[:, :NB, :], in0=attnf[:, :NB, :], scalar=SCALE,
                    in1=bias_all[:, b0:b0 + NB, :], op0=AL.mult, op1=AL.add)
                nc.vector.scalar_tensor_tensor(
                    out=attnf[:, :NB, :], in0=attnf[:, :NB, :], scalar=0.2,
                    in1=attnf[:, :NB, :], op0=AL.mult, op1=AL.max)
                # wva = [V * attn | attn]  (one combined rhs so the two
                # segment-sums collapse into one matmul per block)
                wva = sb2.tile([128, NBMAX, 132], f16, tag="wva")
                nc.scalar.activation(out=wva[:, :NB, 128:132],
                                     in_=attnf[:, :NB, :],
                                     func=mybir.ActivationFunctionType.Exp)
                nc.vector.tensor_tensor(
                    out=wva[:, :NB, 0:128].rearrange("p b (h d) -> p b h d", d=32),
                    in0=kvg[:, :NB, 128:256].rearrange("p b (h d) -> p b h d", d=32),
                    in1=wva[:, :NB, 128:132, None].to_broadcast([128, NB, 4, 32]),
                    op=AL.mult)

                # scatter to nodes: pagg2 = [agg | attn_sum]
                pagg2 = ps.tile([128, 132], f32, tag="pagg")
                for b in range(NB):
                    nc.tensor.matmul(out=pagg2[:], lhsT=oh[:, b, :], rhs=wva[:, b, :],
                                     start=(b == 0), stop=(b == NB - 1))

                # normalize
                sums = sb.tile([128, 4], f32, tag="sums")
                nc.vector.tensor_scalar(out=sums[:], in0=pagg2[:, 128:132],
                                        scalar1=1e-12,
                                        scalar2=None, op0=AL.max)
                rec = sb.tile([128, 4], f32, tag="rec")
                nc.vector.reciprocal(out=rec[:], in_=sums[:])
                aggn = sb.tile([128, 128], f32, tag="aggn")
                nc.vector.tensor_tensor(
                    out=aggn[:].rearrange("p (h d) -> p h d", d=32),
                    in0=pagg2[:, 0:128].rearrange("p (h d) -> p h d", d=32),
                    in1=rec[:, :, None].to_broadcast([128, 4, 32]), op=AL.mult)
                ptr = psb.tile([128, 128], f32, tag="ptrpo")
                nc.tensor.transpose(out=ptr[:], in_=aggn[:], identity=idt[:])
                aggnT = sb.tile([128, 128], f16, tag="aggnT")
                nc.scalar.copy(out=aggnT[:], in_=ptr[:])

                # out = relu(x@Wm1 + aggn@W2 + b2)
                po = psb.tile([128, 128], f32, tag="ptrpo")
                nc.tensor.matmul(out=po[:], lhsT=xtl_sb[:, g * 128:(g + 1) * 128],
                                 rhs=wm1_sb[:], start=True, stop=False)
                nc.tensor.matmul(out=po[:], lhsT=aggnT[:], rhs=w2_sb[:],
                                 start=False, stop=False)
                nc.tensor.matmul(out=po[:], lhsT=ones1[:], rhs=b2_sb[:],
                                 start=False, stop=True)
                osb = sb.tile([128, 128], f16, tag="osb")
                nc.scalar.activation(out=osb[:], in_=po[:],
                                     func=mybir.ActivationFunctionType.Relu)
                nc.sync.dma_start(out=out[g * 128:g * 128 + rows, :],
                                  in_=osb[:rows, :])

    _split_multi_waits(nc, mybir)
    return nc


def kernel(x, edge_index, edge_attr, Wq, Wk, Wv, We, Wo, bo, Wm, bm):
    from concourse.bass_utils import run_bass_kernel_spmd

    x = np.asarray(x, dtype=np.float32)
    edge_attr = np.asarray(edge_attr, dtype=np.float16)
    per_core, nbs, b0s, B = _prep(np.asarray(edge_index), edge_attr)

    key = (tuple(nbs.tolist()), B)
    if key not in _CACHE:
        _CACHE[key] = _build(nbs, b0s, B)
    nc = _CACHE[key]

    xT = x.T  # [128, N]
    Wqkv = np.concatenate(
        [np.asarray(Wq, np.float32), np.asarray(Wk, np.float32),
         np.asarray(Wv, np.float32)], axis=1).astype(np.float16)
    Wm = np.asarray(Wm, np.float32)
    common = dict(
        Wqkv=np.ascontiguousarray(Wqkv),
        Wm1=np.ascontiguousarray(Wm[:128]).astype(np.float16),
        Wm2=np.ascontiguousarray(Wm[128:]).astype(np.float16),
        WoT=np.ascontiguousarray(np.asarray(Wo, np.float32).T).astype(np.float16),
        boc=np.asarray(bo, np.float32).reshape(128, 1).astype(np.float16),
        bmr=np.asarray(bm, np.float32).reshape(1, 128),
        Wef=np.asarray(We, np.float32).reshape(1, 12),
    )
    in_maps = []
    for c in range(NCORES):
        m = dict(common)
        cols = np.zeros((128, NPAD), dtype=np.float16)
        cols[:, :NPC] = xT[:, c * NPC:(c + 1) * NPC]
        m["xtl"] = cols
        m.update(per_core[c])
        in_maps.append(m)

    import time as _time
    _t0 = _time.perf_counter()
    res = run_bass_kernel_spmd(nc, in_maps, core_ids=list(range(NCORES)))
    global _LAST_RESULTS, _LAST_RUN_NS
    _LAST_RUN_NS = int((_time.perf_counter() - _t0) * 1e9)
    _LAST_RESULTS = res
    outs = [res.results[c]["out"] for c in range(NCORES)]
    return np.concatenate(outs, axis=0).astype(np.float32)


_LAST_RESULTS = None
_LAST_RUN_NS = None


# revision 4
# speedup vs baseline: 1.6182x; 1.3984x over previous
"""AttentionSAGEConv on 8 Trainium2 NeuronCores (Bass/Tile).

Strategy (dst-partitioned SPMD, transfer-optimized):
  - The wall-clock metric is dominated by host->device transfer over
    the axon tunnel (and NEFF load), so per-core inputs are minimized:
    each core receives only its 1/8 node slice of x (fp16, 1.6MB),
    uint16 gather indices, uint8 local-dst ids (two layouts), fp16
    edge_attr, and fp16 weights -- ~35MB total across 8 cores vs 277MB
    for the naive replicated-x layout.  The output is fp16 (cast to
    f32 on host).
  - Phase 1 (device): each core projects ONLY its local 6272 nodes to
    Q/K/V (fp16 matmuls).  K|V rows go to a local DRAM table; one
    8-core AllGather over NeuronLink replicates the full fp16 K|V
    table [8*6272, 256] (core-major rows).  Q stays SBUF-resident.
  - Phase 2 (device, per 128-node group): one indirect-DMA gather per
    128-edge block fetches K|V fp16 rows by core-major global src row
    (gathers stay on the default SWDGE queue: spreading them over
    extra named queues costs >1s of NEFF-load wall time for ~zero
    exec gain).  Q rows come from a one-hot PE expansion; the
    transposed one-hot is built directly with a partition-broadcast
    DMA of the host-transposed ldst + is_equal against a per-partition
    iota, not per-block PE transposes.  Per-edge attention edge-major
    on DVE/ACT (QK dot, device edge-bias prepass, leaky relu, exp; the
    global max subtraction cancels in the softmax and is skipped).
    Both segment-sums ride ONE one-hot matmul per block via a combined
    [V*attn | attn] rhs into f32 PSUM, then clamp+reciprocal
    normalization and the fused output
    out = relu(x @ Wm1 + agg_n @ (Wo @ Wm2) + (bo @ Wm2 + bm)).
  Measured relative error 5.8e-4 (vs 2e-2 gate).
"""

import numpy as np

N = 50000
E = 800000
IN_DIM = 128
OUT_DIM = 128
EDGE_DIM = 3
H = 4
HD = 32
SCALE = HD ** -0.5
NCORES = 8
NPC = N // NCORES          # nodes per core = 6250
G = (NPC + 127) // 128     # groups per core = 49
NPAD = G * 128             # padded nodes per core = 6272

_CACHE = {}


def _patch_tile(tile_mod, mybir, ScopedClock):
    """This walrus build allows at most ONE semaphore wait per
    instruction.  Tile's final drain aggregates many waits; replace it
    with a chain of single-wait nops, and post-split every multi-wait
    instruction the Rust scheduler produced."""
    if getattr(tile_mod.TileContext, "_ant_drain_patched", False):
        return

    def _drain_and_barrier(self, tick_clock, wait_clock):
        probe = self.nc.sync.nop(nofuse=True)
        wait_clock.add_sem_waits(probe.ins, ScopedClock({None: tick_clock.global_clock}))
        si = probe.ins.sync_info
        waits = list(si.on_wait) if si is not None and si.on_wait else []
        if len(waits) > 1:
            probe.ins.sync_info = mybir.SyncInfo(on_wait=[waits[0]], on_update=[])
            for w in waits[1:]:
                n = self.nc.sync.nop(nofuse=True)
                n.ins.sync_info = mybir.SyncInfo(on_wait=[w], on_update=[])
        self.nc.sync.drain()
        self.nc.all_engine_barrier()
        popped = self.nc._tile_sem_poison_stack.pop()
        assert popped is self._sem_poison
        self.nc.clear_and_free_semaphores(list(self.sems.allocated().values()))
        self.nc.all_engine_barrier()

    tile_mod.TileContext._drain_and_barrier = _drain_and_barrier
    tile_mod.TileContext._ant_drain_patched = True


def _split_multi_waits(nc, mybir):
    for f in nc.m.functions:
        for blk in f.blocks:
            new = []
            for inst in blk.instructions:
                si = inst.sync_info
                if si is not None and si.on_wait and len(si.on_wait) > 1:
                    waits = list(si.on_wait)
                    for k, w in enumerate(waits[:-1]):
                        new.append(mybir.InstNoOp(
                            name=f"{inst.name}-ws{k}", engine=inst.engine,
                            sync_info=mybir.SyncInfo(on_wait=[w], on_update=[]),
                            bass_nofuse=True))
                    inst.sync_info = mybir.SyncInfo(
                        on_wait=[waits[-1]], on_update=list(si.on_update or []))
                new.append(inst)
            blk.instructions = new


def _prep(edge_index, edge_attr):
    """Host-side index prep.  Returns per-core arrays with one shared
    block structure (NB blocks per group on every core/group).

    srcidx holds CORE-MAJOR global rows into the AllGathered K|V table:
    row = (src // NPC) * NPAD + (src % NPC)."""
    src = np.asarray(edge_index[0], dtype=np.int64)
    dst = np.asarray(edge_index[1], dtype=np.int64)
    src_row = ((src // NPC) * NPAD + (src % NPC)).astype(np.uint16)
    core = dst // NPC
    per_core = []
    counts_all = np.zeros((NCORES, G), dtype=np.int64)
    for c in range(NCORES):
        sel = np.nonzero(core == c)[0]
        d_loc = dst[sel] - c * NPC
        order = np.argsort(d_loc, kind="stable")
        sel = sel[order]
        d_loc = d_loc[order]
        counts = np.bincount(d_loc // 128, minlength=G)
        counts_all[c] = counts
        per_core.append((sel, d_loc, counts))

    # per-group block count = max over cores (SPMD needs per-g uniformity)
    nbs = ((counts_all.max(axis=0) + 127) // 128).astype(int)
    nbs = np.maximum(nbs, 1)
    b0s = np.concatenate([[0], np.cumsum(nbs)]).astype(int)
    B = int(b0s[-1])
    ins = []
    for c in range(NCORES):
        sel, d_loc, counts = per_core[c]
        srcidx = np.zeros((128, B), dtype=np.uint16)
        ldst = np.full((128, B), 255, dtype=np.uint8)
        ea = np.zeros((128, B, 3), dtype=np.float16)
        starts = np.concatenate([[0], np.cumsum(counts)])
        for g in range(G):
            e0, e1 = starts[g], starts[g + 1]
            idxs = sel[e0:e1]
            k = e1 - e0
            slot = np.arange(k)
            b = b0s[g] + slot // 128
            p = slot % 128
            srcidx[p, b] = src_row[idxs]
            ldst[p, b] = (d_loc[e0:e1] - g * 128).astype(np.uint8)
            ea[p, b, :] = edge_attr[idxs]
        ldstT = np.ascontiguousarray(ldst.T).reshape(1, B * 128)
        ins.append(dict(srcidx=srcidx, ldst=ldst, ldstT=ldstT, eat=ea))
    return ins, nbs, b0s, B


def _build(nbs, b0s, B, bufs2=3, bufsps=2):
    import concourse.bass as bass
    import concourse.mybir as mybir
    import concourse.tile as tile
    from concourse.vector_clock import ScopedClock
    from concourse.masks import make_identity

    _patch_tile(tile, mybir, ScopedClock)
    f32 = mybir.dt.float32
    f16 = mybir.dt.float16
    AL = mybir.AluOpType

    nc = bass.Bass(target_bir_lowering=False, num_swdge_queues=4)
    # ---- inputs (per core) ----
    xtl = nc.dram_tensor("xtl", [128, NPAD], f16, kind="ExternalInput")
    Wqkv = nc.dram_tensor("Wqkv", [128, 384], f16, kind="ExternalInput")
    Wm1 = nc.dram_tensor("Wm1", [128, 128], f16, kind="ExternalInput")
    Wm2 = nc.dram_tensor("Wm2", [128, 128], f16, kind="ExternalInput")
    WoT = nc.dram_tensor("WoT", [128, 128], f16, kind="ExternalInput")
    boc = nc.dram_tensor("boc", [128, 1], f16, kind="ExternalInput")
    bmr = nc.dram_tensor("bmr", [1, 128], f32, kind="ExternalInput")
    srcidx = nc.dram_tensor("srcidx", [128, B], mybir.dt.uint16, kind="ExternalInput")
    ldst = nc.dram_tensor("ldst", [128, B], mybir.dt.uint8, kind="ExternalInput")
    ldstT = nc.dram_tensor("ldstT", [1, B * 128], mybir.dt.uint8, kind="ExternalInput")
    eat = nc.dram_tensor("eat", [128, B, 3], f16, kind="ExternalInput")
    Wef = nc.dram_tensor("Wef", [1, 12], f32, kind="ExternalInput")
    out = nc.dram_tensor("out", [NPC, 128], f16, kind="ExternalOutput")
    kvl = nc.dram_tensor("kvl", [NPAD, 256], f16)            # local K|V
    kvt = nc.dram_tensor("kvt", [NCORES * NPAD, 256], f16,
                         addr_space="Shared")                # gathered K|V

    with tile.TileContext(nc) as tc:
        with tc.tile_pool(name="const", bufs=1) as cpool, \
             tc.tile_pool(name="sb", bufs=3) as sb, \
             tc.tile_pool(name="sb2", bufs=bufs2) as sb2, \
             tc.tile_pool(name="ps", bufs=bufsps, space="PSUM") as ps, \
             tc.tile_pool(name="psb", bufs=1, space="PSUM") as psb, \
             tc.tile_pool(name="ps1", bufs=2, space="PSUM") as ps1:

            # ---------- constants / setup ----------
            idt = cpool.tile([128, 128], f32)
            make_identity(nc, idt[:])
            iota16 = cpool.tile([128, 128], f16)
            nc.gpsimd.iota(iota16[:], pattern=[[1, 128]], base=0,
                           channel_multiplier=0,
                           allow_small_or_imprecise_dtypes=True)
            iotac = cpool.tile([128, 1], f16)
            nc.gpsimd.iota(iotac[:], pattern=[[1, 1]], base=0,
                           channel_multiplier=1,
                           allow_small_or_imprecise_dtypes=True)
            xtl_sb = cpool.tile([128, NPAD], f16)
            nc.sync.dma_start(out=xtl_sb[:], in_=xtl[:])
            wqkv_sb = cpool.tile([128, 384], f16)
            nc.sync.dma_start(out=wqkv_sb[:], in_=Wqkv[:])
            wm1_sb = cpool.tile([128, 128], f16)
            nc.sync.dma_start(out=wm1_sb[:], in_=Wm1[:])
            wm2_sb = cpool.tile([128, 128], f16)
            nc.sync.dma_start(out=wm2_sb[:], in_=Wm2[:])
            woT_sb = cpool.tile([128, 128], f16)
            nc.sync.dma_start(out=woT_sb[:], in_=WoT[:])
            bo_sb = cpool.tile([128, 1], f16)
            nc.sync.dma_start(out=bo_sb[:], in_=boc[:])
            bm_sb = cpool.tile([1, 128], f32)
            nc.sync.dma_start(out=bm_sb[:], in_=bmr[:])
            ones1 = cpool.tile([1, 128], f16)
            nc.gpsimd.memset(ones1[:], 1.0)
            ones1f = cpool.tile([1, 128], f32)
            nc.gpsimd.memset(ones1f[:], 1.0)
            wef_sb = cpool.tile([1, 12], f32)
            nc.sync.dma_start(out=wef_sb[:], in_=Wef[:])

            # integer/bias inputs -> working dtypes
            srci_u16 = cpool.tile([128, B], mybir.dt.uint16)
            nc.sync.dma_start(out=srci_u16[:], in_=srcidx[:])
            srcidx_sb = cpool.tile([128, B], mybir.dt.int32)
            nc.vector.tensor_copy(out=srcidx_sb[:], in_=srci_u16[:])
            ldst_u8 = cpool.tile([128, B], mybir.dt.uint8)
            nc.sync.dma_start(out=ldst_u8[:], in_=ldst[:])
            ldst_sb = cpool.tile([128, B], f16)
            nc.vector.tensor_copy(out=ldst_sb[:], in_=ldst_u8[:])
            ea16 = cpool.tile([128, B, 3], f16)
            nc.sync.dma_start(out=ea16[:], in_=eat[:])
            eaf = cpool.tile([128, B, 3], f32)
            nc.scalar.copy(out=eaf[:], in_=ea16[:])

            # W2 = Wo @ Wm2  (WoT supplied pre-transposed from host)
            pw2 = ps1.tile([128, 384], f32, tag="p1p")
            nc.tensor.matmul(out=pw2[:, :128], lhsT=woT_sb[:], rhs=wm2_sb[:], start=True, stop=True)
            w2_sb = cpool.tile([128, 128], f16)
            nc.scalar.copy(out=w2_sb[:], in_=pw2[:, :128])

            # b2 = bo @ Wm2 + bm   [1, 128] fp16
            pb2 = ps1.tile([128, 384], f32, tag="p1p")
            nc.tensor.matmul(out=pb2[:1, :128], lhsT=bo_sb[:], rhs=wm2_sb[:], start=True, stop=True)
            b2_sb = cpool.tile([1, 128], f16)
            nc.vector.tensor_tensor(out=b2_sb[:], in0=pb2[:1, :128], in1=bm_sb[:], op=AL.add)

            # We replicated to all partitions: [128, 12]
            pwe = ps1.tile([128, 384], f32, tag="p1p")
            nc.tensor.matmul(out=pwe[:, :12], lhsT=ones1f[:], rhs=wef_sb[:], start=True, stop=True)
            we_rep = cpool.tile([128, 12], f32)
            nc.scalar.copy(out=we_rep[:], in_=pwe[:, :12])

            # edge bias prepass: bias_all [128, B, 4]
            bias_all = cpool.tile([128, B, 4], f32)
            for h in range(H):
                nc.vector.tensor_scalar(
                    out=bias_all[:, :, h], in0=eaf[:, :, 0],
                    scalar1=we_rep[:, 0 * 4 + h:0 * 4 + h + 1], scalar2=None,
                    op0=AL.mult)
                for j in (1, 2):
                    nc.vector.scalar_tensor_tensor(
                        out=bias_all[:, :, h], in0=eaf[:, :, j],
                        scalar=we_rep[:, j * 4 + h:j * 4 + h + 1],
                        in1=bias_all[:, :, h], op0=AL.mult, op1=AL.add)

            # ---------- phase 1: local Q/K/V; K|V -> DRAM, Q stays SBUF ----------
            qtab = cpool.tile([128, G, 128], f16)
            for g in range(G):
                pq = ps1.tile([128, 384], f32, tag="p1p")
                nc.tensor.matmul(out=pq[:],
                                 lhsT=xtl_sb[:, g * 128:(g + 1) * 128],
                                 rhs=wqkv_sb[:], start=True, stop=True)
                kv16 = sb.tile([128, 256], f16, tag="p1o")
                if g % 2 == 0:
                    nc.vector.tensor_copy(out=qtab[:, g, :], in_=pq[:, 0:128])
                    nc.scalar.copy(out=kv16[:], in_=pq[:, 128:384])
                else:
                    nc.scalar.copy(out=qtab[:, g, :], in_=pq[:, 0:128])
                    nc.vector.tensor_copy(out=kv16[:], in_=pq[:, 128:384])
                nc.sync.dma_start(out=kvl[g * 128:(g + 1) * 128, :], in_=kv16[:])

            # ---------- AllGather K|V across the 8 cores ----------
            nc.gpsimd.collective_compute(
                "AllGather", mybir.AluOpType.bypass,
                replica_groups=[list(range(NCORES))],
                ins=[kvl.ap().opt()], outs=[kvt.ap().opt()])

            # ---------- phase 2 ----------
            NBMAX = int(max(nbs))
            for g in range(G):
                NB = int(nbs[g])
                b0 = int(b0s[g])
                rows = min(128, NPC - g * 128)

                # gathered per-edge K|V for the whole group
                kvg = sb2.tile([128, NBMAX, 256], f16, tag="kvg")
                for b in range(NB):
                    nc.gpsimd.indirect_dma_start(
                        out=kvg[:, b, :], out_offset=None, in_=kvt[:],
                        in_offset=bass.IndirectOffsetOnAxis(
                            ap=srcidx_sb[:, b0 + b:b0 + b + 1], axis=0))

                # one-hot [128e, NB, 128n] for scatter; transposed one-hot
                # [128n, NB, 128e] for Q expansion (built from a broadcast
                # DMA of the host-transposed ldst, not per-block PE
                # transposes)
                oh = sb2.tile([128, NBMAX, 128], f16, tag="oh")
                nc.vector.tensor_tensor(
                    out=oh[:, :NB, :],
                    in0=ldst_sb[:, b0:b0 + NB, None].to_broadcast([128, NB, 128]),
                    in1=iota16[:, None, :].to_broadcast([128, NB, 128]),
                    op=AL.is_equal)
                repu = sb.tile([128, NBMAX * 128], mybir.dt.uint8, tag="repu")
                nc.sync.dma_start(
                    out=repu[:, :NB * 128],
                    in_=ldstT[:, b0 * 128:(b0 + NB) * 128].to_broadcast(
                        [128, NB * 128]))
                rep16 = sb.tile([128, NBMAX * 128], f16, tag="rep16")
                nc.vector.tensor_copy(out=rep16[:, :NB * 128],
                                      in_=repu[:, :NB * 128])
                ohT = sb2.tile([128, NBMAX, 128], f16, tag="ohT")
                nc.vector.tensor_tensor(
                    out=ohT[:, :NB, :],
                    in0=rep16[:, :NB * 128].rearrange("p (b e) -> p b e", e=128),
                    in1=iotac[:, :, None].to_broadcast([128, NB, 128]),
                    op=AL.is_equal)
                pk = sb2.tile([128, NBMAX, 128], f32, tag="pk")
                for b4 in range(0, NB, 4):
                    nb4 = min(4, NB - b4)
                    pqe = psb.tile([128, 4, 128], f32, tag="pqe")
                    for j in range(nb4):
                        b = b4 + j
                        nc.tensor.matmul(out=pqe[:, j, :], lhsT=ohT[:, b, :],
                                         rhs=qtab[:, g, :], start=True, stop=True)
                    nc.vector.tensor_tensor(out=pk[:, b4:b4 + nb4, :],
                                            in0=pqe[:, :nb4, :],
                                            in1=kvg[:, b4:b4 + nb4, 0:128], op=AL.mult)
                attnf = sb2.tile([128, NBMAX, 4], f32, tag="attnf")
                nc.vector.tensor_reduce(
                    out=attnf[:, :NB, :],
                    in_=pk[:, :NB, :].rearrange("p b (h d) -> p (b h) d", d=32),
                    axis=mybir.AxisListType.X, op=AL.add)
                # z = attn*SCALE + bias ; leaky = max(z, 0.2 z) ; exp
                nc.vector.scalar_tensor_tensor(
                    out=attnf[:, :NB, :], in0=attnf[:, :NB, :], scalar=SCALE,
                    in1=bias_all[:, b0:b0 + NB, :], op0=AL.mult, op1=AL.add)
                nc.vector.scalar_tensor_tensor(
                    out=attnf[:, :NB, :], in0=attnf[:, :NB, :], scalar=0.2,
                    in1=attnf[:, :NB, :], op0=AL.mult, op1=AL.max)
                # wva = [V * attn | attn]  (one combined rhs so the two
                # segment-sums collapse into one matmul per block)
                wva = sb2.tile([128, NBMAX, 132], f16, tag="wva")
                nc.scalar.activation(out=wva[:, :NB, 128:132],
                                     in_=attnf[:, :NB, :],
                                     func=mybir.ActivationFunctionType.Exp)
                nc.vector.tensor_tensor(
                    out=wva[:, :NB, 0:128].rearrange("p b (h d) -> p b h d", d=32),
                    in0=kvg[:, :NB, 128:256].rearrange("p b (h d) -> p b h d", d=32),
                    in1=wva[:, :NB, 128:132, None].to_broadcast([128, NB, 4, 32]),
                    op=AL.mult)

                # scatter to nodes: pagg2 = [agg | attn_sum]
                pagg2 = ps.tile([128, 132], f32, tag="pagg")
                for b in range(NB):
                    nc.tensor.matmul(out=pagg2[:], lhsT=oh[:, b, :], rhs=wva[:, b, :],
                                     start=(b == 0), stop=(b == NB - 1))

                # normalize
                sums = sb.tile([128, 4], f32, tag="sums")
                nc.vector.tensor_scalar(out=sums[:], in0=pagg2[:, 128:132],
                                        scalar1=1e-12,
                                        scalar2=None, op0=AL.max)
                rec = sb.tile([128, 4], f32, tag="rec")
                nc.vector.reciprocal(out=rec[:], in_=sums[:])
                aggn = sb.tile([128, 128], f32, tag="aggn")
                nc.vector.tensor_tensor(
                    out=aggn[:].rearrange("p (h d) -> p h d", d=32),
                    in0=pagg2[:, 0:128].rearrange("p (h d) -> p h d", d=32),
                    in1=rec[:, :, None].to_broadcast([128, 4, 32]), op=AL.mult)
                ptr = psb.tile([128, 128], f32, tag="ptrpo")
                nc.tensor.transpose(out=ptr[:], in_=aggn[:], identity=idt[:])
                aggnT = sb.tile([128, 128], f16, tag="aggnT")
                nc.scalar.copy(out=aggnT[:], in_=ptr[:])

                # out = relu(x@Wm1 + aggn@W2 + b2)
                po = psb.tile([128, 128], f32, tag="ptrpo")
                nc.tensor.matmul(out=po[:], lhsT=xtl_sb[:, g * 128:(g + 1) * 128],
                                 rhs=wm1_sb[:], start=True, stop=False)
                nc.tensor.matmul(out=po[:], lhsT=aggnT[:], rhs=w2_sb[:],
                                 start=False, stop=False)
                nc.tensor.matmul(out=po[:], lhsT=ones1[:], rhs=b2_sb[:],
                                 start=False, stop=True)
                osb = sb.tile([128, 128], f16, tag="osb")
                nc.scalar.activation(out=osb[:], in_=po[:],
                                     func=mybir.ActivationFunctionType.Relu)
                nc.sync.dma_start(out=out[g * 128:g * 128 + rows, :],
                                  in_=osb[:rows, :])

    _split_multi_waits(nc, mybir)
    return nc


def kernel(x, edge_index, edge_attr, Wq, Wk, Wv, We, Wo, bo, Wm, bm):
    from concourse.bass_utils import run_bass_kernel_spmd

    x = np.asarray(x, dtype=np.float32)
    edge_attr = np.asarray(edge_attr, dtype=np.float16)
    per_core, nbs, b0s, B = _prep(np.asarray(edge_index), edge_attr)

    key = (tuple(nbs.tolist()), B)
    if key not in _CACHE:
        _CACHE[key] = _build(nbs, b0s, B)
    nc = _CACHE[key]

    xT = x.T  # [128, N]
    Wqkv = np.concatenate(
        [np.asarray(Wq, np.float32), np.asarray(Wk, np.float32),
         np.asarray(Wv, np.float32)], axis=1).astype(np.float16)
    Wm = np.asarray(Wm, np.float32)
    common = dict(
        Wqkv=np.ascontiguousarray(Wqkv),
        Wm1=np.ascontiguousarray(Wm[:128]).astype(np.float16),
        Wm2=np.ascontiguousarray(Wm[128:]).astype(np.float16),
        WoT=np.ascontiguousarray(np.asarray(Wo, np.float32).T).astype(np.float16),
        boc=np.asarray(bo, np.float32).reshape(128, 1).astype(np.float16),
        bmr=np.asarray(bm, np.float32).reshape(1, 128),
        Wef=np.asarray(We, np.float32).reshape(1, 12),
    )
    in_maps = []
    for c in range(NCORES):
        m = dict(common)
        cols = np.zeros((128, NPAD), dtype=np.float16)
        cols[:, :NPC] = xT[:, c * NPC:(c + 1) * NPC]
        m["xtl"] = cols
        m.update(per_core[c])
        in_maps.append(m)

    # Touch the devices once so PJRT client/tunnel handshake happens here
    # rather than inside the run.
    import jax
    jax.block_until_ready(jax.device_put(
        np.zeros(NCORES, np.float32), jax.devices()[0]))

    import time as _time
    _t0 = _time.perf_counter()
    res = run_bass_kernel_spmd(nc, in_maps, core_ids=list(range(NCORES)))
    global _LAST_RESULTS, _LAST_RUN_NS
    _LAST_RUN_NS = int((_time.perf_counter() - _t0) * 1e9)
    _LAST_RESULTS = res
    outs = [res.results[c]["out"] for c in range(NCORES)]
    return np.concatenate(outs, axis=0).astype(np.float32)


_LAST_RESULTS = None
_LAST_RUN_NS = None


# revision 9
# speedup vs baseline: 1.6610x; 1.0264x over previous
"""AttentionSAGEConv on 8 Trainium2 NeuronCores (Bass/Tile).

Strategy (dst-partitioned SPMD, transfer-optimized):
  - The wall-clock metric is dominated by host->device transfer over
    the axon tunnel (and NEFF load), so per-core inputs are minimized:
    each core receives only its 1/8 node slice of x (fp16, 1.6MB),
    uint16 gather indices, uint8 local-dst ids (two layouts), fp16
    edge_attr, and fp16 weights -- ~35MB total across 8 cores vs 277MB
    for the naive replicated-x layout.  The output is fp16 (cast to
    f32 on host).
  - Phase 1 (device): each core projects ONLY its local 6272 nodes to
    Q/K/V (fp16 matmuls).  K|V rows go to a local DRAM table; one
    8-core AllGather over NeuronLink replicates the full fp16 K|V
    table [8*6272, 256] (core-major rows).  Q stays SBUF-resident.
  - Phase 2 (device, per 128-node group): one indirect-DMA gather per
    128-edge block fetches K|V fp16 rows by core-major global src row
    (gathers stay on the default SWDGE queue: spreading them over
    extra named queues costs >1s of NEFF-load wall time for ~zero
    exec gain).  Q rows come from a one-hot PE expansion; the
    transposed one-hot is built directly with a partition-broadcast
    DMA of the host-transposed ldst + is_equal against a per-partition
    iota, not per-block PE transposes.  Per-edge attention edge-major
    on DVE/ACT (QK dot, device edge-bias prepass, leaky relu, exp; the
    global max subtraction cancels in the softmax and is skipped).
    Both segment-sums ride ONE one-hot matmul per block via a combined
    [V*attn | attn] rhs into f32 PSUM, then clamp+reciprocal
    normalization and the fused output
    out = relu(x @ Wm1 + agg_n @ (Wo @ Wm2) + (bo @ Wm2 + bm)).
  Measured relative error 5.8e-4 (vs 2e-2 gate).
"""

import numpy as np

N = 50000
E = 800000
IN_DIM = 128
OUT_DIM = 128
EDGE_DIM = 3
H = 4
HD = 32
SCALE = HD ** -0.5
NCORES = 8
NPC = N // NCORES          # nodes per core = 6250
G = (NPC + 127) // 128     # groups per core = 49
NPAD = G * 128             # padded nodes per core = 6272

_CACHE = {}


def _patch_tile(tile_mod, mybir, ScopedClock):
    """This walrus build allows at most ONE semaphore wait per
    instruction.  Tile's final drain aggregates many waits; replace it
    with a chain of single-wait nops, and post-split every multi-wait
    instruction the Rust scheduler produced."""
    if getattr(tile_mod.TileContext, "_ant_drain_patched", False):
        return

    def _drain_and_barrier(self, tick_clock, wait_clock):
        probe = self.nc.sync.nop(nofuse=True)
        wait_clock.add_sem_waits(probe.ins, ScopedClock({None: tick_clock.global_clock}))
        si = probe.ins.sync_info
        waits = list(si.on_wait) if si is not None and si.on_wait else []
        if len(waits) > 1:
            probe.ins.sync_info = mybir.SyncInfo(on_wait=[waits[0]], on_update=[])
            for w in waits[1:]:
                n = self.nc.sync.nop(nofuse=True)
                n.ins.sync_info = mybir.SyncInfo(on_wait=[w], on_update=[])
        self.nc.sync.drain()
        self.nc.all_engine_barrier()
        popped = self.nc._tile_sem_poison_stack.pop()
        assert popped is self._sem_poison
        self.nc.clear_and_free_semaphores(list(self.sems.allocated().values()))
        self.nc.all_engine_barrier()

    tile_mod.TileContext._drain_and_barrier = _drain_and_barrier
    tile_mod.TileContext._ant_drain_patched = True


def _split_multi_waits(nc, mybir):
    for f in nc.m.functions:
        for blk in f.blocks:
            new = []
            for inst in blk.instructions:
                si = inst.sync_info
                if si is not None and si.on_wait and len(si.on_wait) > 1:
                    waits = list(si.on_wait)
                    for k, w in enumerate(waits[:-1]):
                        new.append(mybir.InstNoOp(
                            name=f"{inst.name}-ws{k}", engine=inst.engine,
                            sync_info=mybir.SyncInfo(on_wait=[w], on_update=[]),
                            bass_nofuse=True))
                    inst.sync_info = mybir.SyncInfo(
                        on_wait=[waits[-1]], on_update=list(si.on_update or []))
                new.append(inst)
            blk.instructions = new


def _prep(edge_index, edge_attr):
    """Host-side index prep.  Returns per-core arrays with one shared
    block structure (NB blocks per group on every core/group).

    srcidx holds CORE-MAJOR global rows into the AllGathered K|V table:
    row = (src // NPC) * NPAD + (src % NPC)."""
    src = np.asarray(edge_index[0], dtype=np.int64)
    dst = np.asarray(edge_index[1], dtype=np.int64)
    src_row = ((src // NPC) * NPAD + (src % NPC)).astype(np.uint16)
    core = dst // NPC
    per_core = []
    counts_all = np.zeros((NCORES, G), dtype=np.int64)
    for c in range(NCORES):
        sel = np.nonzero(core == c)[0]
        d_loc = dst[sel] - c * NPC
        order = np.argsort(d_loc, kind="stable")
        sel = sel[order]
        d_loc = d_loc[order]
        counts = np.bincount(d_loc // 128, minlength=G)
        counts_all[c] = counts
        per_core.append((sel, d_loc, counts))

    # per-group block count = max over cores (SPMD needs per-g uniformity)
    nbs = ((counts_all.max(axis=0) + 127) // 128).astype(int)
    nbs = np.maximum(nbs, 1)
    b0s = np.concatenate([[0], np.cumsum(nbs)]).astype(int)
    B = int(b0s[-1])
    ins = []
    for c in range(NCORES):
        sel, d_loc, counts = per_core[c]
        srcidx = np.zeros((128, B), dtype=np.uint16)
        ldst = np.full((128, B), 255, dtype=np.uint8)
        ea = np.zeros((128, B, 3), dtype=np.float16)
        starts = np.concatenate([[0], np.cumsum(counts)])
        for g in range(G):
            e0, e1 = starts[g], starts[g + 1]
            idxs = sel[e0:e1]
            k = e1 - e0
            slot = np.arange(k)
            b = b0s[g] + slot // 128
            p = slot % 128
            srcidx[p, b] = src_row[idxs]
            ldst[p, b] = (d_loc[e0:e1] - g * 128).astype(np.uint8)
            ea[p, b, :] = edge_attr[idxs]
        ldstT = np.ascontiguousarray(ldst.T).reshape(1, B * 128)
        ins.append(dict(srcidx=srcidx, ldst=ldst, ldstT=ldstT, eat=ea))
    return ins, nbs, b0s, B


def _build(nbs, b0s, B, bufs2=3, bufsps=2):
    import concourse.bass as bass
    import concourse.mybir as mybir
    import concourse.tile as tile
    from concourse.vector_clock import ScopedClock
    from concourse.masks import make_identity

    _patch_tile(tile, mybir, ScopedClock)
    f32 = mybir.dt.float32
    f16 = mybir.dt.float16
    AL = mybir.AluOpType

    nc = bass.Bass(target_bir_lowering=False, num_swdge_queues=4)
    # ---- inputs (per core) ----
    xtl = nc.dram_tensor("xtl", [128, NPAD], f16, kind="ExternalInput")
    Wqkv = nc.dram_tensor("Wqkv", [128, 384], f16, kind="ExternalInput")
    Wm1 = nc.dram_tensor("Wm1", [128, 128], f16, kind="ExternalInput")
    Wm2 = nc.dram_tensor("Wm2", [128, 128], f16, kind="ExternalInput")
    WoT = nc.dram_tensor("WoT", [128, 128], f16, kind="ExternalInput")
    boc = nc.dram_tensor("boc", [128, 1], f16, kind="ExternalInput")
    bmr = nc.dram_tensor("bmr", [1, 128], f32, kind="ExternalInput")
    srcidx = nc.dram_tensor("srcidx", [128, B], mybir.dt.uint16, kind="ExternalInput")
    ldst = nc.dram_tensor("ldst", [128, B], mybir.dt.uint8, kind="ExternalInput")
    ldstT = nc.dram_tensor("ldstT", [1, B * 128], mybir.dt.uint8, kind="ExternalInput")
    eat = nc.dram_tensor("eat", [128, B, 3], f16, kind="ExternalInput")
    Wef = nc.dram_tensor("Wef", [1, 12], f32, kind="ExternalInput")
    out = nc.dram_tensor("out", [NPC, 128], f16, kind="ExternalOutput")
    kvl = nc.dram_tensor("kvl", [NPAD, 256], f16)            # local K|V
    kvt = nc.dram_tensor("kvt", [NCORES * NPAD, 256], f16,
                         addr_space="Shared")                # gathered K|V

    with tile.TileContext(nc) as tc:
        with tc.tile_pool(name="const", bufs=1) as cpool, \
             tc.tile_pool(name="sb", bufs=3) as sb, \
             tc.tile_pool(name="sb2", bufs=bufs2) as sb2, \
             tc.tile_pool(name="ps", bufs=bufsps, space="PSUM") as ps, \
             tc.tile_pool(name="psb", bufs=1, space="PSUM") as psb, \
             tc.tile_pool(name="ps1", bufs=2, space="PSUM") as ps1:

            # ---------- constants / setup ----------
            idt = cpool.tile([128, 128], f32)
            make_identity(nc, idt[:])
            iota16 = cpool.tile([128, 128], f16)
            nc.gpsimd.iota(iota16[:], pattern=[[1, 128]], base=0,
                           channel_multiplier=0,
                           allow_small_or_imprecise_dtypes=True)
            iotac = cpool.tile([128, 1], f16)
            nc.gpsimd.iota(iotac[:], pattern=[[1, 1]], base=0,
                           channel_multiplier=1,
                           allow_small_or_imprecise_dtypes=True)
            xtl_sb = cpool.tile([128, NPAD], f16)
            nc.sync.dma_start(out=xtl_sb[:], in_=xtl[:])
            wqkv_sb = cpool.tile([128, 384], f16)
            nc.sync.dma_start(out=wqkv_sb[:], in_=Wqkv[:])
            wm1_sb = cpool.tile([128, 128], f16)
            nc.sync.dma_start(out=wm1_sb[:], in_=Wm1[:])
            wm2_sb = cpool.tile([128, 128], f16)
            nc.sync.dma_start(out=wm2_sb[:], in_=Wm2[:])
            woT_sb = cpool.tile([128, 128], f16)
            nc.sync.dma_start(out=woT_sb[:], in_=WoT[:])
            bo_sb = cpool.tile([128, 1], f16)
            nc.sync.dma_start(out=bo_sb[:], in_=boc[:])
            bm_sb = cpool.tile([1, 128], f32)
            nc.sync.dma_start(out=bm_sb[:], in_=bmr[:])
            ones1 = cpool.tile([1, 128], f16)
            nc.gpsimd.memset(ones1[:], 1.0)
            ones1f = cpool.tile([1, 128], f32)
            nc.gpsimd.memset(ones1f[:], 1.0)
            wef_sb = cpool.tile([1, 12], f32)
            nc.sync.dma_start(out=wef_sb[:], in_=Wef[:])

            # integer/bias inputs -> working dtypes
            srci_u16 = cpool.tile([128, B], mybir.dt.uint16)
            nc.sync.dma_start(out=srci_u16[:], in_=srcidx[:])
            srcidx_sb = cpool.tile([128, B], mybir.dt.int32)
            nc.vector.tensor_copy(out=srcidx_sb[:], in_=srci_u16[:])
            ldst_u8 = cpool.tile([128, B], mybir.dt.uint8)
            nc.sync.dma_start(out=ldst_u8[:], in_=ldst[:])
            ldst_sb = cpool.tile([128, B], f16)
            nc.vector.tensor_copy(out=ldst_sb[:], in_=ldst_u8[:])
            ea16 = cpool.tile([128, B, 3], f16)
            nc.sync.dma_start(out=ea16[:], in_=eat[:])
            eaf = cpool.tile([128, B, 3], f32)
            nc.scalar.copy(out=eaf[:], in_=ea16[:])

            # W2 = Wo @ Wm2  (WoT supplied pre-transposed from host)
            pw2 = ps1.tile([128, 384], f32, tag="p1p")
            nc.tensor.matmul(out=pw2[:, :128], lhsT=woT_sb[:], rhs=wm2_sb[:], start=True, stop=True)
            w2_sb = cpool.tile([128, 128], f16)
            nc.scalar.copy(out=w2_sb[:], in_=pw2[:, :128])

            # b2 = bo @ Wm2 + bm   [1, 128] fp16
            pb2 = ps1.tile([128, 384], f32, tag="p1p")
            nc.tensor.matmul(out=pb2[:1, :128], lhsT=bo_sb[:], rhs=wm2_sb[:], start=True, stop=True)
            b2_sb = cpool.tile([1, 128], f16)
            nc.vector.tensor_tensor(out=b2_sb[:], in0=pb2[:1, :128], in1=bm_sb[:], op=AL.add)

            # We replicated to all partitions: [128, 12]
            pwe = ps1.tile([128, 384], f32, tag="p1p")
            nc.tensor.matmul(out=pwe[:, :12], lhsT=ones1f[:], rhs=wef_sb[:], start=True, stop=True)
            we_rep = cpool.tile([128, 12], f32)
            nc.scalar.copy(out=we_rep[:], in_=pwe[:, :12])

            # edge bias prepass: bias_all [128, B, 4]
            bias_all = cpool.tile([128, B, 4], f32)
            for h in range(H):
                nc.vector.tensor_scalar(
                    out=bias_all[:, :, h], in0=eaf[:, :, 0],
                    scalar1=we_rep[:, 0 * 4 + h:0 * 4 + h + 1], scalar2=None,
                    op0=AL.mult)
                for j in (1, 2):
                    nc.vector.scalar_tensor_tensor(
                        out=bias_all[:, :, h], in0=eaf[:, :, j],
                        scalar=we_rep[:, j * 4 + h:j * 4 + h + 1],
                        in1=bias_all[:, :, h], op0=AL.mult, op1=AL.add)

            # ---------- phase 1: local Q/K/V; K|V -> DRAM, Q stays SBUF ----------
            qtab = cpool.tile([128, G, 128], f16)
            for g in range(G):
                pq = ps1.tile([128, 384], f32, tag="p1p")
                nc.tensor.matmul(out=pq[:],
                                 lhsT=xtl_sb[:, g * 128:(g + 1) * 128],
                                 rhs=wqkv_sb[:], start=True, stop=True)
                kv16 = sb.tile([128, 256], f16, tag="p1o")
                if g % 2 == 0:
                    nc.vector.tensor_copy(out=qtab[:, g, :], in_=pq[:, 0:128])
                    nc.scalar.copy(out=kv16[:], in_=pq[:, 128:384])
                else:
                    nc.scalar.copy(out=qtab[:, g, :], in_=pq[:, 0:128])
                    nc.vector.tensor_copy(out=kv16[:], in_=pq[:, 128:384])
                nc.sync.dma_start(out=kvl[g * 128:(g + 1) * 128, :], in_=kv16[:])

            # ---------- AllGather K|V across the 8 cores ----------
            nc.gpsimd.collective_compute(
                "AllGather", mybir.AluOpType.bypass,
                replica_groups=[list(range(NCORES))],
                ins=[kvl.ap().opt()], outs=[kvt.ap().opt()])

            # ---------- phase 2 ----------
            NBMAX = int(max(nbs))
            for g in range(G):
                NB = int(nbs[g])
                b0 = int(b0s[g])
                rows = min(128, NPC - g * 128)

                # gathered per-edge K|V for the whole group
                kvg = sb2.tile([128, NBMAX, 256], f16, tag="kvg")
                for b in range(NB):
                    nc.gpsimd.indirect_dma_start(
                        out=kvg[:, b, :], out_offset=None, in_=kvt[:],
                        in_offset=bass.IndirectOffsetOnAxis(
                            ap=srcidx_sb[:, b0 + b:b0 + b + 1], axis=0))

                # one-hot [128e, NB, 128n] for scatter; transposed one-hot
                # [128n, NB, 128e] for Q expansion (built from a broadcast
                # DMA of the host-transposed ldst, not per-block PE
                # transposes)
                oh = sb2.tile([128, NBMAX, 128], f16, tag="oh")
                nc.vector.tensor_tensor(
                    out=oh[:, :NB, :],
                    in0=ldst_sb[:, b0:b0 + NB, None].to_broadcast([128, NB, 128]),
                    in1=iota16[:, None, :].to_broadcast([128, NB, 128]),
                    op=AL.is_equal)
                repu = sb.tile([128, NBMAX * 128], mybir.dt.uint8, tag="repu")
                nc.sync.dma_start(
                    out=repu[:, :NB * 128],
                    in_=ldstT[:, b0 * 128:(b0 + NB) * 128].to_broadcast(
                        [128, NB * 128]))
                rep16 = sb.tile([128, NBMAX * 128], f16, tag="rep16")
                nc.vector.tensor_copy(out=rep16[:, :NB * 128],
                                      in_=repu[:, :NB * 128])
                ohT = sb2.tile([128, NBMAX, 128], f16, tag="ohT")
                nc.vector.tensor_tensor(
                    out=ohT[:, :NB, :],
                    in0=rep16[:, :NB * 128].rearrange("p (b e) -> p b e", e=128),
                    in1=iotac[:, :, None].to_broadcast([128, NB, 128]),
                    op=AL.is_equal)
                pk = sb2.tile([128, NBMAX, 128], f32, tag="pk")
                for b4 in range(0, NB, 4):
                    nb4 = min(4, NB - b4)
                    pqe = psb.tile([128, 4, 128], f32, tag="pqe")
                    for j in range(nb4):
                        b = b4 + j
                        nc.tensor.matmul(out=pqe[:, j, :], lhsT=ohT[:, b, :],
                                         rhs=qtab[:, g, :], start=True, stop=True)
                    nc.vector.tensor_tensor(out=pk[:, b4:b4 + nb4, :],
                                            in0=pqe[:, :nb4, :],
                                            in1=kvg[:, b4:b4 + nb4, 0:128], op=AL.mult)
                attnf = sb2.tile([128, NBMAX, 4], f32, tag="attnf")
                nc.vector.tensor_reduce(
                    out=attnf[:, :NB, :],
                    in_=pk[:, :NB, :].rearrange("p b (h d) -> p (b h) d", d=32),
                    axis=mybir.AxisListType.X, op=AL.add)
                # z = attn*SCALE + bias ; leaky = max(z, 0.2 z) ; exp
                nc.vector.scalar_tensor_tensor(
                    out=attnf[:, :NB, :], in0=attnf[:, :NB, :], scalar=SCALE,
                    in1=bias_all[:, b0:b0 + NB, :], op0=AL.mult, op1=AL.add)
                nc.vector.scalar_tensor_tensor(
                    out=attnf[:, :NB, :], in0=attnf[:, :NB, :], scalar=0.2,
                    in1=attnf[:, :NB, :], op0=AL.mult, op1=AL.max)
                nc.vector.tensor_scalar(
                    out=attnf[:, :NB, :], in0=attnf[:, :NB, :], scalar1=-6.0,
                    scalar2=None, op0=AL.add)
                # wva = [V * attn | attn]  (one combined rhs so the two
                # segment-sums collapse into one matmul per block).
                # exp is shifted by a constant: any per-group constant
                # cancels exactly in the per-node softmax (a node's edges
                # all live in one group), and without it V*exp(z) overflows
                # fp16 when max-logit exceeds ~9.7 (observed 10.5 on other
                # seeds -> NaN).  exp(z-6) keeps V*attn <= ~2e3 for z <= 13.
                wva = sb2.tile([128, NBMAX, 132], f16, tag="wva")
                nc.scalar.activation(out=wva[:, :NB, 128:132],
                                     in_=attnf[:, :NB, :],
                                     func=mybir.ActivationFunctionType.Exp)
                nc.vector.tensor_tensor(
                    out=wva[:, :NB, 0:128].rearrange("p b (h d) -> p b h d", d=32),
                    in0=kvg[:, :NB, 128:256].rearrange("p b (h d) -> p b h d", d=32),
                    in1=wva[:, :NB, 128:132, None].to_broadcast([128, NB, 4, 32]),
                    op=AL.mult)

                # scatter to nodes: pagg2 = [agg | attn_sum]
                pagg2 = ps.tile([128, 132], f32, tag="pagg")
                for b in range(NB):
                    nc.tensor.matmul(out=pagg2[:], lhsT=oh[:, b, :], rhs=wva[:, b, :],
                                     start=(b == 0), stop=(b == NB - 1))

                # normalize
                sums = sb.tile([128, 4], f32, tag="sums")
                nc.vector.tensor_scalar(out=sums[:], in0=pagg2[:, 128:132],
                                        scalar1=1e-12,
                                        scalar2=None, op0=AL.max)
                rec = sb.tile([128, 4], f32, tag="rec")
                nc.vector.reciprocal(out=rec[:], in_=sums[:])
                aggn = sb.tile([128, 128], f32, tag="aggn")
                nc.vector.tensor_tensor(
                    out=aggn[:].rearrange("p (h d) -> p h d", d=32),
                    in0=pagg2[:, 0:128].rearrange("p (h d) -> p h d", d=32),
                    in1=rec[:, :, None].to_broadcast([128, 4, 32]), op=AL.mult)
                ptr = psb.tile([128, 128], f32, tag="ptrpo")
                nc.tensor.transpose(out=ptr[:], in_=aggn[:], identity=idt[:])
                aggnT = sb.tile([128, 128], f16, tag="aggnT")
                nc.scalar.copy(out=aggnT[:], in_=ptr[:])

                # out = relu(x@Wm1 + aggn@W2 + b2)
                po = psb.tile([128, 128], f32, tag="ptrpo")
                nc.tensor.matmul(out=po[:], lhsT=xtl_sb[:, g * 128:(g + 1) * 128],
                                 rhs=wm1_sb[:], start=True, stop=False)
                nc.tensor.matmul(out=po[:], lhsT=aggnT[:], rhs=w2_sb[:],
                                 start=False, stop=False)
                nc.tensor.matmul(out=po[:], lhsT=ones1[:], rhs=b2_sb[:],
                                 start=False, stop=True)
                osb = sb.tile([128, 128], f16, tag="osb")
                nc.scalar.activation(out=osb[:], in_=po[:],
                                     func=mybir.ActivationFunctionType.Relu)
                nc.sync.dma_start(out=out[g * 128:g * 128 + rows, :],
                                  in_=osb[:rows, :])

    _split_multi_waits(nc, mybir)
    return nc


def kernel(x, edge_index, edge_attr, Wq, Wk, Wv, We, Wo, bo, Wm, bm):
    from concourse.bass_utils import run_bass_kernel_spmd

    # Establish the PJRT client / axon tunnel handshake concurrently with
    # host-side prep+build (the handshake is network-bound and can stall
    # for seconds; prep/build is CPU-bound).
    import threading

    def _warm():
        try:
            import jax
            jax.block_until_ready(jax.device_put(
                np.zeros(NCORES, np.float32), jax.devices()[0]))
        except Exception:
            pass

    warm_t = threading.Thread(target=_warm, daemon=True)
    warm_t.start()

    x = np.asarray(x, dtype=np.float32)
    edge_attr = np.asarray(edge_attr, dtype=np.float16)
    per_core, nbs, b0s, B = _prep(np.asarray(edge_index), edge_attr)

    key = (tuple(nbs.tolist()), B)
    if key not in _CACHE:
        _CACHE[key] = _build(nbs, b0s, B)
    nc = _CACHE[key]

    xT = x.T  # [128, N]
    Wqkv = np.concatenate(
        [np.asarray(Wq, np.float32), np.asarray(Wk, np.float32),
         np.asarray(Wv, np.float32)], axis=1).astype(np.float16)
    Wm = np.asarray(Wm, np.float32)
    common = dict(
        Wqkv=np.ascontiguousarray(Wqkv),
        Wm1=np.ascontiguousarray(Wm[:128]).astype(np.float16),
        Wm2=np.ascontiguousarray(Wm[128:]).astype(np.float16),
        WoT=np.ascontiguousarray(np.asarray(Wo, np.float32).T).astype(np.float16),
        boc=np.asarray(bo, np.float32).reshape(128, 1).astype(np.float16),
        bmr=np.asarray(bm, np.float32).reshape(1, 128),
        Wef=np.asarray(We, np.float32).reshape(1, 12),
    )
    in_maps = []
    for c in range(NCORES):
        m = dict(common)
        cols = np.zeros((128, NPAD), dtype=np.float16)
        cols[:, :NPC] = xT[:, c * NPC:(c + 1) * NPC]
        m["xtl"] = cols
        m.update(per_core[c])
        in_maps.append(m)

    warm_t.join()

    import time as _time
    _t0 = _time.perf_counter()
    res = run_bass_kernel_spmd(nc, in_maps, core_ids=list(range(NCORES)))
    global _LAST_RESULTS, _LAST_RUN_NS
    _LAST_RUN_NS = int((_time.perf_counter() - _t0) * 1e9)
    _LAST_RESULTS = res
    outs = [res.results[c]["out"] for c in range(NCORES)]
    return np.concatenate(outs, axis=0).astype(np.float32)


_LAST_RESULTS = None
_LAST_RUN_NS = None
